# revision 41
# baseline (speedup 1.0000x reference)
"""Trainium2 Bass kernel for nn_BudgetAwareHalting (B=16, S=4096, H=1024, D=512).

Reference semantics recap (see the original nn.Module):
  comp_mean[s] = mean_b sigmoid(relu(states @ cW1 + cb1) @ cW2 + cb2)   # complexity net
  output       = states                                                 # passthrough
  steps_used / rb_final come from a sequential budget scan whose branch
  `steps = where(halt, MIN_STEPS, max_for)` is provably halt-independent for
  these constants: max_for = clip(trunc(rb/(S-t+1)), 1, 8) == 1 for every t
  (rb <= 128 <= S-t+1 while rb > 0, and trunc <= 0 once rb <= 0), so steps == 1
  at every scan step no matter what the data is.  The halt-net GEMM is dead
  code.  The scan recursion is reproduced exactly on the host in fp32 (with an
  assert that the halt-independence invariant holds), which leaves the
  complexity-net GEMM as the only device work.

Device strategy (8 NeuronCores, data-parallel over batch, 2 batches/core):
  - host: transpose+tile each core's shard to xT[blk, p, ko, t] so every
    block DMA reads contiguous 16 KiB per partition (the PE needs the
    contraction dim H on SBUF partitions; DMAing that layout from the
    natural [tok, H] would be 4-byte-strided and ~10x slower).
  - v2 program (default; biases are zero in the reference setup): per
    128-token psum tile, 8 accumulating float32r matmuls (full 1 cyc/row PE
    rate at N=512), ScalarE relu psum->sbuf, VectorE mul+reduce against a
    host-replicated cW2 row -> logit [128,1], ScalarE sigmoid(+cb2) into a
    [128, 32] accumulator; one 16 KiB DMA out per core.
  - v1 fallback (general, any biases): D on psum partitions, relu+bias via
    ScalarE, second layer as 4 M=1 matmuls accumulated in PSUM.
  - ramp-up blocks and w1 are DMA'd per 256 KiB k-chunk into separate tiles
    (Tile deps are whole-tile) so the first matmul fires ~10 us earlier.
  - host: sum the 8 per-core partials, divide by B.

Measured on trn2 (8 cores, NTFF): ~150 us, comp_mean max rel err ~1.7e-5
vs the fp32 jax reference (float32r rounds mantissas; exceeds fp32 GEMM at
4 cyc/row = ~440 us PE, and bf16 which is less accurate at the same speed).
"""

import os
import numpy as np

# ---------------------------------------------------------------- constants
TOTAL_BUDGET = 128.0
MAX_STEPS = 8
MIN_STEPS = 1

B, S, H = 16, 4096, 1024
D = H // 2
P = 128
N_CORES = 8
B_PER_CORE = B // N_CORES          # 2
TOK = B_PER_CORE * S               # 8192 tokens per core
BLK = 512                          # tokens per block (= fp32 moving-operand max)
KCH = H // P                       # 8 contraction chunks
DCH = D // P                       # 4 D chunks

_CACHE = {}


# ---------------------------------------------------------------- bass program
def _build_program(matmul_dtype="float32r", nblk=None, do_logit=True,
                   logit_accum=True, hpsum_bufs=5, lpsum_bufs=2):
    import concourse.bass as bass
    import concourse.tile as tile
    from concourse import bacc, mybir

    f32 = mybir.dt.float32
    mm_dt = getattr(mybir.dt, matmul_dtype)

    nc = bacc.Bacc(None, target_bir_lowering=False, debug=False)

    NBLK_IO = (nblk if nblk is not None else TOK // BLK)
    # xT is pre-tiled on the host: xT[j, p, ko, t] = states_flat[j*BLK+t, ko*P+p]
    # so each partition's slice of a block DMA is one contiguous 16 KiB run.
    xT = nc.dram_tensor("xT", [NBLK_IO, P, KCH, BLK], mm_dt,
                        kind="ExternalInput").ap()
    w1 = nc.dram_tensor("w1", [H, D], mm_dt, kind="ExternalInput").ap()
    w2 = nc.dram_tensor("w2", [D], mm_dt, kind="ExternalInput").ap()
    b1 = nc.dram_tensor("b1", [D], f32, kind="ExternalInput").ap()
    b2 = nc.dram_tensor("b2", [1, 1], f32, kind="ExternalInput").ap()
    comp = nc.dram_tensor("comp", [1, S], f32, kind="ExternalOutput").ap()

    NBLK = nblk if nblk is not None else TOK // BLK   # 16
    BLK_PER_B = S // BLK           # 8 blocks per local batch

    with tile.TileContext(nc) as tc:
        with (
            tc.tile_pool(name="const", bufs=1) as cpool,
            tc.tile_pool(name="xpool", bufs=3) as xpool,
            tc.tile_pool(name="hpool", bufs=2) as hpool,
            tc.tile_pool(name="spool", bufs=2) as spool,
            tc.tile_pool(name="hpsum", bufs=hpsum_bufs, space="PSUM") as ppool,
            tc.tile_pool(name="lpsum", bufs=lpsum_bufs, space="PSUM") as lpool,
        ):
            # stationary weights / biases
            w1_sb = cpool.tile([P, KCH, D], mm_dt)
            nc.sync.dma_start(out=w1_sb[:], in_=w1.rearrange("(ko p) d -> p ko d", p=P))
            w2_sb = cpool.tile([P, DCH], mm_dt)
            nc.sync.dma_start(out=w2_sb[:], in_=w2.rearrange("(o p) -> p o", p=P))
            b1_sb = cpool.tile([P, DCH], f32)
            nc.sync.dma_start(out=b1_sb[:], in_=b1.rearrange("(o p) -> p o", p=P))
            b2_sb = cpool.tile([1, 1], f32)
            nc.sync.dma_start(out=b2_sb[:], in_=b2)

            # partial sum over this core's local batches, indexed by s
            acc = cpool.tile([1, S], f32)

            for j in range(NBLK):
                b_loc = j // BLK_PER_B
                s0 = (j % BLK_PER_B) * BLK

                x_sb = xpool.tile([P, KCH, BLK], mm_dt)
                nc.sync.dma_start(out=x_sb[:], in_=xT[j])

                relu_sb = hpool.tile([P, DCH, BLK], mm_dt)
                for d in range(DCH):
                    h_ps = ppool.tile([P, BLK], f32)
                    for k in range(KCH):
                        nc.tensor.matmul(
                            h_ps[:],
                            w1_sb[:, k, d * P:(d + 1) * P],
                            x_sb[:, k, :],
                            start=(k == 0),
                            stop=(k == KCH - 1),
                        )
                    # relu(h + b1) while moving PSUM -> SBUF
                    nc.scalar.activation(
                        out=relu_sb[:, d, :],
                        in_=h_ps[:],
                        func=mybir.ActivationFunctionType.Relu,
                        bias=b1_sb[:, d:d + 1],
                    )

                if not do_logit:
                    if j == NBLK - 1:
                        nc.vector.tensor_copy(out=acc[0:1, :BLK],
                                              in_=relu_sb[0:1, 0, :])
                    continue

                if logit_accum:
                    logit_ps = lpool.tile([1, BLK], f32)
                    for d in range(DCH):
                        nc.tensor.matmul(
                            logit_ps[:],
                            w2_sb[:, d:d + 1],
                            relu_sb[:, d, :],
                            start=(d == 0),
                            stop=(d == DCH - 1),
                        )
                else:
                    lg = []
                    for d in range(DCH):
                        lp_d = lpool.tile([1, BLK], f32, tag="lg")
                        nc.tensor.matmul(
                            lp_d[:], w2_sb[:, d:d + 1], relu_sb[:, d, :],
                            start=True, stop=True,
                        )
                        lg.append(lp_d)
                    logit_ps = spool.tile([1, BLK], f32, tag="lsum")
                    nc.vector.tensor_add(out=logit_ps[:], in0=lg[0][:], in1=lg[1][:])
                    nc.vector.tensor_add(out=logit_ps[:], in0=logit_ps[:], in1=lg[2][:])
                    nc.vector.tensor_add(out=logit_ps[:], in0=logit_ps[:], in1=lg[3][:])

                if b_loc == 0:
                    # first local batch: write sigmoid straight into acc
                    nc.scalar.activation(
                        out=acc[0:1, s0:s0 + BLK],
                        in_=logit_ps[:],
                        func=mybir.ActivationFunctionType.Sigmoid,
                        bias=b2_sb[0:1, 0:1],
                    )
                else:
                    sig_sb = spool.tile([1, BLK], f32)
                    nc.scalar.activation(
                        out=sig_sb[:],
                        in_=logit_ps[:],
                        func=mybir.ActivationFunctionType.Sigmoid,
                        bias=b2_sb[0:1, 0:1],
                    )
                    nc.vector.tensor_add(
                        out=acc[0:1, s0:s0 + BLK],
                        in0=acc[0:1, s0:s0 + BLK],
                        in1=sig_sb[:],
                    )

            nc.sync.dma_start(out=comp, in_=acc[0:1, :])

    nc.compile()
    return nc


def _build_program_v2(matmul_dtype="float32r", use_b1=False, hpsum_bufs=7,
                      out2d=True, use_ttr=False, nblk=None,
                      hpool_bufs=4, zpool_bufs=3):
    # use_ttr=True (InstTensorTensorReduce) crashes TRN2 at runtime in this
    # toolchain (NRT_EXEC_UNIT_UNRECOVERABLE) -- keep the mul+reduce pair.
    """Orientation-flipped variant: tokens on PSUM partitions.

    First GEMM: lhsT = x chunk [128h, 128tok], rhs = w1 [128h, 512D] ->
    psum h [128tok, 512D].  Second layer runs off the PE entirely:
    ACT relu psum->sbuf, DVE tensor_tensor_reduce against host-replicated w2
    -> logit [128,1], ACT sigmoid(+b2) -> acc column.  PE does only the 512
    unavoidable GEMM matmuls.  use_b1=True inserts a DVE add of the
    host-replicated cb1 before the relu (general path; cb1 is zero in the
    reference setup).
    """
    import concourse.tile as tile
    from concourse import bacc, mybir

    f32 = mybir.dt.float32
    mm_dt = getattr(mybir.dt, matmul_dtype)

    nc = bacc.Bacc(None, target_bir_lowering=False, debug=False)

    NBLK = nblk if nblk is not None else TOK // BLK   # 16
    BLK_PER_B = S // BLK           # 8
    TSUB = BLK // P                # 4 token subtiles per block
    G = S // P                     # 32 acc columns

    xT = nc.dram_tensor("xT", [NBLK, P, KCH, BLK], mm_dt,
                        kind="ExternalInput").ap()
    # w1t is host-tiled: w1t[p, ko, d] = cW1[ko*P + p, d] (contiguous 16 KiB
    # per partition)
    w1 = nc.dram_tensor("w1t", [P, KCH, D], mm_dt, kind="ExternalInput").ap()
    w2r = nc.dram_tensor("w2r", [P, D], f32, kind="ExternalInput").ap()
    b2r = nc.dram_tensor("b2r", [P, 1], f32, kind="ExternalInput").ap()
    if use_b1:
        b1r = nc.dram_tensor("b1r", [P, D], f32, kind="ExternalInput").ap()
    if out2d:
        comp = nc.dram_tensor("comp", [P, G], f32, kind="ExternalOutput").ap()
    else:
        comp = nc.dram_tensor("comp", [1, S], f32, kind="ExternalOutput").ap()

    with tile.TileContext(nc) as tc:
        with (
            tc.tile_pool(name="const", bufs=1) as cpool,
            tc.tile_pool(name="xpool", bufs=3) as xpool,
            tc.tile_pool(name="xchunk", bufs=3 * KCH) as xcpool,
            tc.tile_pool(name="hpool", bufs=hpool_bufs) as hpool,
            tc.tile_pool(name="scratch", bufs=zpool_bufs) as zpool,
            tc.tile_pool(name="spool", bufs=4) as spool,
            tc.tile_pool(name="hpsum", bufs=hpsum_bufs, space="PSUM") as ppool,
        ):
            # Tile dependency tracking is whole-tile, so the ramp-up blocks and
            # w1 use one tile PER k-chunk: matmuls fire as each 256 KiB chunk
            # lands instead of waiting for whole 2 MiB blocks.
            SPLIT_BLOCKS = 3
            w1_ks = []
            for k in range(KCH):
                xk = xcpool.tile([P, BLK], mm_dt, tag="xc")
                nc.sync.dma_start(out=xk[:], in_=xT[0, :, k, :])
                wk = cpool.tile([P, D], mm_dt, name=f"w1_{k}")
                nc.sync.dma_start(out=wk[:], in_=w1[:, k, :])
                if k == 0:
                    x_chunks = {0: [xk]}
                else:
                    x_chunks[0].append(xk)
                w1_ks.append(wk)
            for jj in range(1, SPLIT_BLOCKS):
                x_chunks[jj] = []
                for k in range(KCH):
                    xk = xcpool.tile([P, BLK], mm_dt, tag="xc")
                    nc.sync.dma_start(out=xk[:], in_=xT[jj, :, k, :])
                    x_chunks[jj].append(xk)
            w2r_sb = cpool.tile([P, D], f32)
            nc.sync.dma_start(out=w2r_sb[:], in_=w2r)
            b2r_sb = cpool.tile([P, 1], f32)
            nc.sync.dma_start(out=b2r_sb[:], in_=b2r)
            if use_b1:
                b1r_sb = cpool.tile([P, D], f32)
                nc.sync.dma_start(out=b1r_sb[:], in_=b1r)

            acc = cpool.tile([P, G], f32)

            for j in range(NBLK):
                b_loc = j // BLK_PER_B
                if j < SPLIT_BLOCKS:
                    x_sb = None
                else:
                    x_sb = xpool.tile([P, KCH, BLK], mm_dt, tag="x")
                    nc.sync.dma_start(out=x_sb[:], in_=xT[j])

                for m in range(TSUB):
                    h_ps = ppool.tile([P, D], f32)
                    for k in range(KCH):
                        lhsT = (x_chunks[j][k][:, m * P:(m + 1) * P]
                                if j < SPLIT_BLOCKS
                                else x_sb[:, k, m * P:(m + 1) * P])
                        nc.tensor.matmul(
                            h_ps[:],
                            lhsT,
                            w1_ks[k][:],
                            start=(k == 0),
                            stop=(k == KCH - 1),
                        )
                    relu_sb = hpool.tile([P, D], f32)
                    if use_b1:
                        tmp = zpool.tile([P, D], f32, tag="b1tmp")
                        nc.vector.tensor_add(out=tmp[:], in0=h_ps[:], in1=b1r_sb[:])
                        nc.scalar.activation(
                            out=relu_sb[:], in_=tmp[:],
                            func=mybir.ActivationFunctionType.Relu)
                    else:
                        nc.scalar.activation(
                            out=relu_sb[:], in_=h_ps[:],
                            func=mybir.ActivationFunctionType.Relu)

                    logit = spool.tile([P, 1], f32, tag="logit")
                    if use_ttr:
                        dead = zpool.tile([P, D], f32, tag="dead")
                        nc.vector.tensor_tensor_reduce(
                            out=dead[:],
                            in0=relu_sb[:],
                            in1=w2r_sb[:],
                            scale=1.0,
                            scalar=0.0,
                            op0=mybir.AluOpType.mult,
                            op1=mybir.AluOpType.add,
                            accum_out=logit[:],
                        )
                    else:
                        prod = zpool.tile([P, D], f32, tag="dead")
                        nc.vector.tensor_mul(out=prod[:], in0=relu_sb[:],
                                             in1=w2r_sb[:])
                        nc.vector.tensor_reduce(
                            out=logit[:], in_=prod[:],
                            axis=mybir.AxisListType.X,
                            op=mybir.AluOpType.add)

                    g = (j % BLK_PER_B) * TSUB + m
                    if b_loc == 0:
                        nc.scalar.activation(
                            out=acc[:, g:g + 1], in_=logit[:],
                            func=mybir.ActivationFunctionType.Sigmoid,
                            bias=b2r_sb[:, 0:1])
                    else:
                        sig = spool.tile([P, 1], f32, tag="sig")
                        nc.scalar.activation(
                            out=sig[:], in_=logit[:],
                            func=mybir.ActivationFunctionType.Sigmoid,
                            bias=b2r_sb[:, 0:1])
                        nc.vector.tensor_add(
                            out=acc[:, g:g + 1], in0=acc[:, g:g + 1], in1=sig[:])

            if out2d:
                nc.sync.dma_start(out=comp, in_=acc[:])
            else:
                nc.sync.dma_start(
                    out=comp.rearrange("x (g p) -> p (x g)", p=P), in_=acc[:])

    nc.compile()
    return nc


def _get_program(matmul_dtype="float32r"):
    key = ("prog", matmul_dtype)
    if key not in _CACHE:
        _CACHE[key] = _build_program(matmul_dtype)
    return _CACHE[key]


# ---------------------------------------------------------------- host helpers
def _tile_shard(shard):
    """[TOK, H] C-contiguous -> [NBLK, P, KCH, BLK] DMA-tiled layout:
    out[j, p, ko, t] = shard[j*BLK + t, ko*P + p] (block-local transpose)."""
    nblk = shard.shape[0] // BLK
    v = shard.reshape(nblk, BLK, KCH, P)
    out = np.empty((nblk, P, KCH, BLK), dtype=shard.dtype)
    for j in range(nblk):            # 2 MiB panels keep the working set cached
        out[j] = v[j].transpose(2, 1, 0)
    return out


def _host_scan(S_len):
    """Reproduce the reference budget scan in fp32 host arithmetic.

    Valid because steps == where(halt, MIN_STEPS, max_for) is halt-independent
    whenever max_for == MIN_STEPS, which the assert below checks at every step.
    """
    rb = np.float32(TOTAL_BUDGET)
    steps = np.empty(S_len, np.int32)
    for t in range(S_len):
        denom = np.float32(S_len - t + 1)
        # bf*MAX_STEPS == rb/denom exactly (the /8 then *8 are exact in fp32)
        mf = int(np.trunc(np.float32(rb / denom)))
        mf = min(max(mf, MIN_STEPS), MAX_STEPS)
        assert mf == MIN_STEPS, (
            "halt-independence invariant violated; the halt net would be live"
        )
        steps[t] = MIN_STEPS
        rb = np.float32(rb - np.float32(MIN_STEPS))
    return steps, rb


LAST_EXEC_TIME_NS = None


def _install_ntff_hook_shim():
    """Provide antenv.axon_hooks if the image lacks it, so trace=True can
    capture NTFF profiles via libaxon_pjrt.so (same ctypes ABI the axon boot
    script uses). No-op if the real module or the .so is unavailable."""
    import sys
    try:
        import antenv.axon_hooks  # noqa: F401
        return
    except ImportError:
        pass
    import contextlib
    import ctypes
    import types
    so_path = "/opt/axon/libaxon_pjrt.so"
    if not os.path.exists(so_path):
        return
    try:
        lib = ctypes.CDLL(so_path)
    except OSError:
        return
    if not hasattr(lib, "axon_start_nrt_profile"):
        return
    lib.axon_start_nrt_profile.argtypes = [
        ctypes.POINTER(ctypes.c_int64), ctypes.c_size_t]
    lib.axon_start_nrt_profile.restype = ctypes.c_int64
    lib.axon_stop_nrt_profile.argtypes = [ctypes.c_char_p]
    lib.axon_stop_nrt_profile.restype = ctypes.c_int64

    @contextlib.contextmanager
    def _hook(output_dir, device_ids):
        import jax
        jax.devices()
        if device_ids:
            ids = (ctypes.c_int64 * len(device_ids))(*device_ids)
            rc = lib.axon_start_nrt_profile(ids, len(device_ids))
        else:
            rc = lib.axon_start_nrt_profile(None, 0)
        if rc != 0:
            raise RuntimeError(f"axon_start_nrt_profile rc={rc}")
        try:
            yield
        finally:
            n = lib.axon_stop_nrt_profile(str(output_dir).encode())
            print(f"ntff profile: {n} file(s) written to {output_dir}")

    mod = types.ModuleType("antenv.axon_hooks")
    mod.get_axon_ntff_profile_hook = lambda: _hook
    mod.set_axon_ntff_profile_hook = lambda h: None
    sys.modules["antenv.axon_hooks"] = mod


# ---------------------------------------------------------------- entry point
def kernel(states, hW1, hb1, hW2, hb2, cW1, cb1, cW2, cb2):
    global LAST_EXEC_TIME_NS
    from concourse.bass_utils import run_bass_kernel_spmd

    states = np.asarray(states)
    assert states.shape == (B, S, H) and states.dtype == np.float32

    w1 = np.ascontiguousarray(np.asarray(cW1, np.float32))
    w2 = np.ascontiguousarray(np.asarray(cW2, np.float32).reshape(D))
    b1 = np.ascontiguousarray(np.asarray(cb1, np.float32).reshape(D))
    b2 = np.ascontiguousarray(np.asarray(cb2, np.float32).reshape(1, 1))

    use_v2 = not np.any(b1)
    if use_v2:
        key = ("prog_v2",)
        if key not in _CACHE:
            _CACHE[key] = _build_program_v2()
        nc = _CACHE[key]
        w1t = np.ascontiguousarray(w1.reshape(KCH, P, D).transpose(1, 0, 2))
        w2r = np.ascontiguousarray(np.repeat(w2[None, :], P, axis=0))
        b2r = np.full((P, 1), b2.ravel()[0], np.float32)
        core_feed = {"w1t": w1t, "w2r": w2r, "b2r": b2r}
    else:
        nc = _get_program()
        core_feed = {"w1": w1, "w2": w2, "b1": b1, "b2": b2}

    in_maps = []
    for c in range(N_CORES):
        shard = states[c * B_PER_CORE:(c + 1) * B_PER_CORE].reshape(TOK, H)
        in_maps.append({"xT": _tile_shard(shard), **core_feed})

    trace = bool(int(os.environ.get("BAH_TRACE", "0")))
    if trace:
        _install_ntff_hook_shim()
    res = None
    for attempt in range(3):
        try:
            res = run_bass_kernel_spmd(nc, in_maps, list(range(N_CORES)),
                                       trace=trace)
            break
        except Exception as e:
            # device faults have been observed to be transient; retry
            print(f"kernel: attempt {attempt} failed: {type(e).__name__}: "
                  f"{str(e)[:200]}")
            if attempt == 2:
                raise
    LAST_EXEC_TIME_NS = res.exec_time_ns

    if use_v2:
        # comp comes back [P, G] with s = g*P + p
        parts = np.stack([np.asarray(r["comp"], np.float32)
                          .reshape(P, S // P).T.ravel() for r in res.results])
    else:
        parts = np.stack([np.asarray(r["comp"], np.float32).reshape(S)
                          for r in res.results])
    comp_mean = (parts.sum(axis=0, dtype=np.float32)
                 * np.float32(1.0 / B)).astype(np.float32)

    steps_used, rb = _host_scan(S)
    rb_mean = np.float32(rb)

    return states, steps_used, comp_mean, rb_mean
